# revision 25
# baseline (speedup 1.0000x reference)
"""Llama3 attention prefill kernel for 8 Trainium2 NeuronCores.

Sharding: tensor-parallel over heads. Core c owns Q heads 4c..4c+3 and KV
head c (GQA group), plus the matching wqkv columns / wo rows. Each core
computes a partial output y_c = attn_c @ wo_c; the host sums the partials.

Per-core pipeline (all inside one TileContext):
  1. qkvT = w_shard^T @ x  computed directly in transposed layout
     (lhsT = w chunk, rhs = xT chunk streamed from DRAM as f16).
     Host pre-transposes/casts x and permutes q/k weight columns so RoPE
     pairs live in partition blocks [0:64] / [64:128].
  2. RoPE applied in the transposed layout: partner halves obtained with a
     block-swap permutation matmul, then 3 elementwise f16 DVE ops.
  3. Causal flash attention with transposed scores: S^T = K_j^T x Q_group
     per (k-block j, 512-wide q group) -> exp gives P^T directly (no P
     transposes), row sums via gpsimd adds + a ones-vector matmul,
     PV accumulates out^T per q group, normalization fused into the
     PSUM eviction multiply.
  4. y^T = wo_shard^T @ out^T per q group (N=512 matmuls), f16 DMA out.
"""

import os
import sys

for _p in ("/opt/trn_rl_repo", "/root/.axon_site/_ro/trn_rl_repo"):
    if os.path.isdir(_p) and _p not in sys.path:
        sys.path.insert(0, _p)

import numpy as np

S = 2048
H = 4096
HD = 128
NQ = 4            # q heads per core
MQKV = 768        # per-core qkv columns: 512 q + 128 k + 128 v
N_CORES = 8
KC = H // 128     # 32 contraction chunks for qkv
KT = S // 128     # 16 pos tiles
NG = S // 512     # 4 q groups of 512 positions
SCALE = 1.0 / float(np.sqrt(HD))

_CACHE = {}
LAST_RESULTS = None


def _build():
    import concourse.tile as tile
    from concourse import bacc, bass_isa, mybir
    from concourse.masks import make_identity

    f32 = mybir.dt.float32
    f16 = mybir.dt.float16
    Exp = mybir.ActivationFunctionType.Exp

    nc = bacc.Bacc("TRN2", target_bir_lowering=False, debug=False)

    xT_ap = nc.dram_tensor("xT", [H, S], f16, kind="ExternalInput").ap()
    w_ap = nc.dram_tensor("w", [H, MQKV], f16, kind="ExternalInput").ap()
    wo_ap = nc.dram_tensor("wo", [NQ * HD, H], f16, kind="ExternalInput").ap()
    cs_ap = nc.dram_tensor("cs2", [128, S], f16, kind="ExternalInput").ap()
    sn_ap = nc.dram_tensor("sn2", [128, S], f16, kind="ExternalInput").ap()
    dm_ap = nc.dram_tensor("dmask", [128, 128], f16, kind="ExternalInput").ap()
    sw_ap = nc.dram_tensor("swapm", [128, 128], f16, kind="ExternalInput").ap()
    yT_ap = nc.dram_tensor("yT", [H, S], f16, kind="ExternalOutput").ap()

    # DRAM views for batched DMA: [p, chunk, col]
    xT_v = xT_ap.rearrange("(a p) s -> p a s", p=128)      # [128, 32, 2048]
    w_v = w_ap.rearrange("(a p) c -> p a c", p=128)        # [128, 32, 768]
    wo_v = wo_ap.rearrange("(a p) c -> p a c", p=128)      # [128, 4, 4096]
    yT_v = yT_ap.rearrange("(a p) s -> p a s", p=128)      # [128, 32, 2048]

    with tile.TileContext(nc) as tc:
        from contextlib import ExitStack

        with ExitStack() as ctx:
            const = ctx.enter_context(tc.tile_pool(name="const", bufs=1))
            ident = const.tile([128, 128], f16)
            make_identity(nc, ident[:])
            dmask = const.tile([128, 128], f16)
            swapm = const.tile([128, 128], f16)

            # resident tensors
            res = ctx.enter_context(tc.tile_pool(name="res", bufs=1))
            qT_sb = res.tile([128, NQ, S], f16, name="qT_sb")
            kT_sb = res.tile([128, S], f16, name="kT_sb")
            v_sb = res.tile([128, KT, 128], f16, name="v_sb")
            outT_sb = res.tile([128, NQ, S], f16, name="outT_sb")
            cs_sb = res.tile([128, S], f16, name="cs_sb")
            sn_sb = res.tile([128, S], f16, name="sn_sb")

            w_pool_cm = tc.tile_pool(name="w_pool", bufs=1, side="right")
            w_pool = w_pool_cm.__enter__()
            w_sb = w_pool.tile([128, KC, MQKV], f16, name="w_sb")

            xb_pool_cm = tc.tile_pool(name="xb", bufs=2)
            xb_pool = xb_pool_cm.__enter__()

            # streaming pools (phase 1)
            ep_cm = tc.tile_pool(name="ep", bufs=4)
            ep = ep_cm.__enter__()
            rp_cm = tc.tile_pool(name="rp", bufs=4)
            rp = rp_cm.__enter__()

            ps1_cm = tc.tile_pool(name="ps_qkv", bufs=6, space="PSUM")
            ps1 = ps1_cm.__enter__()
            ps1c_cm = tc.tile_pool(name="ps_vt", bufs=1, space="PSUM")
            ps1c = ps1c_cm.__enter__()

            # ---- phase 1: qkv^T projection + rope (transposed layout) ----
            # Interleave the first w / x loads in kc-chunks of 8 so the
            # first matmul chain starts after ~7us of DMA, not ~30us.
            xbufs = [xb_pool.tile([128, KC, 512], f16, tag="xb",
                                  name=f"xb{st}") for st in range(NG)]
            _edges = [0, 2, 4, 6, 8, 12, 16, 20, 24, 28, 32]
            for _a, _b in zip(_edges[:-1], _edges[1:]):
                ksl = slice(_a, _b)
                nc.sync.dma_start(out=w_sb[:, ksl, :], in_=w_v[:, ksl, :])
                nc.sync.dma_start(out=xbufs[0][:, ksl, :],
                                  in_=xT_v[:, ksl, 0:512])
            nc.sync.dma_start(out=swapm[:], in_=sw_ap[:, :])
            nc.sync.dma_start(out=cs_sb[:], in_=cs_ap[:, :])
            nc.sync.dma_start(out=sn_sb[:], in_=sn_ap[:, :])
            nc.sync.dma_start(out=dmask[:], in_=dm_ap[:, :])
            for st in range(1, NG):
                nc.sync.dma_start(
                    out=xbufs[st][:],
                    in_=xT_v[:, :, st * 512:(st + 1) * 512])

            for st in range(NG):
                xb = xbufs[st]
                sl = slice(st * 512, (st + 1) * 512)
                # kq-outer ordering: all 6 head-columns accumulate in 6 PSUM
                # banks per 8-kc chunk, so the first supertile's compute
                # keeps pace with the streaming w/x loads.
                qpss = [ps1.tile([128, 512], f32, tag="qkv",
                                 name=f"qps{st}_{c6}") for c6 in range(6)]
                for kq in range(8):
                    for c6 in range(6):
                        for kc in range(kq * 4, kq * 4 + 4):
                            nc.tensor.matmul(
                                qpss[c6][:],
                                lhsT=w_sb[:, kc, c6 * 128:(c6 + 1) * 128],
                                rhs=xb[:, kc, :],
                                start=(kc == 0), stop=(kc == KC - 1))
                for c6 in range(6):
                    qps = qpss[c6]
                    E = ep.tile([128, 512], f16, tag="E")
                    nc.scalar.copy(out=E[:], in_=qps[:])
                    if c6 == 5:
                        # v: transpose to natural [pos, d] tiles
                        for q in range(4):
                            vt = ps1c.tile([128, 128], f16, tag="vt")
                            nc.tensor.transpose(
                                vt[:], E[:, q * 128:(q + 1) * 128], ident[:])
                            nc.scalar.copy(
                                out=v_sb[:, st * 4 + q, :], in_=vt[:])
                    else:
                        # rope: partner half via two half-partition copies
                        Esw = ep.tile([128, 512], f16, tag="Esw")
                        nc.vector.tensor_copy(out=Esw[0:64, :],
                                              in_=E[64:128, :])
                        nc.vector.tensor_copy(out=Esw[64:128, :],
                                              in_=E[0:64, :])
                        t1 = rp.tile([128, 512], f16, tag="t1")
                        nc.vector.tensor_mul(t1[:], E[:], cs_sb[:, sl])
                        t2 = rp.tile([128, 512], f16, tag="t2")
                        nc.vector.tensor_mul(t2[:], Esw[:], sn_sb[:, sl])
                        dst = (qT_sb[:, c6, sl] if c6 < 4 else kT_sb[:, sl])
                        nc.vector.tensor_add(dst, t1[:], t2[:])

            # ---- phase boundary: free phase-1 pools (LIFO), load wo ----
            ps1c_cm.__exit__(None, None, None)
            ps1_cm.__exit__(None, None, None)
            rp_cm.__exit__(None, None, None)
            ep_cm.__exit__(None, None, None)
            xb_pool_cm.__exit__(None, None, None)
            w_pool_cm.__exit__(None, None, None)

            wo_pool = ctx.enter_context(
                tc.tile_pool(name="wo_pool", bufs=1, side="right"))
            wo_sb = wo_pool.tile([128, NQ, H], f16, name="wo_sb")
            nc.sync.dma_start(out=wo_sb[:], in_=wo_v[:, :, :])

            # phase 2+3 pools
            pp = ctx.enter_context(tc.tile_pool(name="pp", bufs=6))
            rr = ctx.enter_context(tc.tile_pool(name="rr", bufs=3))
            lp = ctx.enter_context(tc.tile_pool(name="lp", bufs=4))
            yp = ctx.enter_context(tc.tile_pool(name="yp", bufs=3))

            ps_s = ctx.enter_context(
                tc.tile_pool(name="ps_s", bufs=4, space="PSUM"))
            ps_o = ctx.enter_context(
                tc.tile_pool(name="ps_o", bufs=2, space="PSUM"))
            ps_y = ctx.enter_context(
                tc.tile_pool(name="ps_y", bufs=2, space="PSUM"))

            # ---- phase 2: causal flash attention (transposed scores),
            # with y^T chunks of the previous group interleaved to keep PE
            # busy through each head's softmax-denominator tail ----
            def y_chunk(gy, y4):
                gysl = slice(gy * 512, (gy + 1) * 512)
                ybig = yp.tile([128, 4, 512], f16, tag="ybig")
                for yi in range(4):
                    ym = y4 * 4 + yi
                    yps = ps_y.tile([128, 512], f32, tag="yps")
                    for kc in range(NQ):
                        nc.tensor.matmul(
                            yps[:],
                            lhsT=wo_sb[:, kc, ym * 128:(ym + 1) * 128],
                            rhs=outT_sb[:, kc, gysl],
                            start=(kc == 0), stop=(kc == NQ - 1))
                    nc.vector.tensor_copy(out=ybig[:, yi, :], in_=yps[:])
                nc.sync.dma_start(
                    out=yT_v[:, y4 * 4:y4 * 4 + 4, gysl], in_=ybig[:])

            def attn_tail(g, h, ops, R):
                # softmax denominators on Pool/DVE only: replicated
                # partition-sum of R, reciprocal, then normalization fused
                # into the out^T PSUM eviction
                gsl = slice(g * 512, (g + 1) * 512)
                l_bc = lp.tile([128, 512], f32, tag="lbc")
                nc.gpsimd.partition_all_reduce(
                    l_bc[:], R[:], channels=128,
                    reduce_op=bass_isa.ReduceOp.add)
                rbc = lp.tile([128, 512], f16, tag="rbc_sb")
                with nc.allow_low_precision(reason="1/l fits f16"):
                    nc.vector.reciprocal(rbc[:], l_bc[:])
                nc.vector.tensor_mul(outT_sb[:, h, gsl], ops[:], rbc[:])

            pending = None
            ycnt = 0
            for g in range(NG):
                for h in range(NQ):
                    ops = ps_o.tile([128, 512], f32, tag="ops")
                    R = rr.tile([128, 512], f16, tag="R")
                    jmax = 4 * g + 3
                    for j in range(jmax + 1):
                        ing = (j // 4 == g)
                        c0 = (j - 4 * g) * 128 if ing else 0
                        sps = ps_s.tile([128, 512], f32, tag="sps")
                        nc.tensor.matmul(
                            sps[:, c0:],
                            lhsT=kT_sb[:, j * 128:(j + 1) * 128],
                            rhs=qT_sb[:, h, g * 512 + c0:(g + 1) * 512],
                            start=True, stop=True)
                        P = pp.tile([128, 512], f16, tag="P")
                        nc.scalar.activation(
                            P[:, c0:], sps[:, c0:], Exp, scale=SCALE)
                        if ing:
                            nc.vector.tensor_mul(
                                P[:, c0:c0 + 128], P[:, c0:c0 + 128],
                                dmask[:])
                        if j == 0:
                            nc.vector.tensor_copy(out=R[:], in_=P[:])
                        else:
                            nc.vector.tensor_add(R[:, c0:], R[:, c0:],
                                                 P[:, c0:])
                        nc.tensor.matmul(
                            ops[:, c0:], lhsT=v_sb[:, j, :], rhs=P[:, c0:],
                            start=(j == 0), stop=(j == jmax))
                    # software-pipelined: the previous head's softmax tail
                    # (Pool/DVE only) lands behind this head's j-loop; y^T
                    # chunks of the previous group fill PE slack.  None are
                    # emitted at h=0 since they would stall on the group
                    # boundary tail that finishes outT of group g-1.
                    if pending is not None:
                        attn_tail(*pending)
                    pending = (g, h, ops, R)
                    if g >= 1:
                        for _ in range((0, 3, 3, 2)[h]):
                            y_chunk(g - 1, ycnt)
                            ycnt += 1
                ycnt = 0
            attn_tail(*pending)

            # ---- final y^T chunks for the last group; the last four
            # column-tiles stream out individually so the kernel tail is a
            # single evict+DMA, not four serial ones ----
            for y4 in range(7):
                y_chunk(NG - 1, y4)
            lgsl = slice((NG - 1) * 512, NG * 512)
            for ym in range(28, 32):
                yps = ps_y.tile([128, 512], f32, tag="yps")
                for kc in range(NQ):
                    nc.tensor.matmul(
                        yps[:],
                        lhsT=wo_sb[:, kc, ym * 128:(ym + 1) * 128],
                        rhs=outT_sb[:, kc, lgsl],
                        start=(kc == 0), stop=(kc == NQ - 1))
                ysm = yp.tile([128, 512], f16, tag="ysm")
                nc.vector.tensor_copy(out=ysm[:], in_=yps[:])
                nc.sync.dma_start(out=yT_v[:, ym:ym + 1, lgsl], in_=ysm[:])

    nc.compile()
    return nc


def _get_nc():
    if "nc" not in _CACHE:
        _CACHE["nc"] = _build()
    return _CACHE["nc"]


def _prep_inputs(x, rope_cache, wqkv, wo):
    x2 = np.asarray(x, np.float32).reshape(S, H)
    xT = np.ascontiguousarray(x2.T.astype(np.float16))          # [H, S]

    rc = np.asarray(rope_cache, np.float32)                      # [S, 64, 2]
    c = rc[:, :, 0].T.astype(np.float16)                         # [64, S]
    s = rc[:, :, 1].T.astype(np.float16)
    cs2 = np.ascontiguousarray(np.vstack([c, c]))                # [128, S]
    sn2 = np.ascontiguousarray(np.vstack([-s, s]))

    # transposed causal mask (multiplicative): keep k <= q
    kk = np.arange(128)
    dmask = (kk[:, None] <= kk[None, :]).astype(np.float16)
    swapm = np.zeros((128, 128), np.float16)
    swapm[kk, (kk + 64) % 128] = 1.0

    # rope pair permutation within each 128-wide head: evens then odds
    perm = np.concatenate([np.arange(0, 128, 2), np.arange(1, 128, 2)])

    wq = np.asarray(wqkv, np.float32)
    wo_f = np.asarray(wo, np.float32)

    in_maps = []
    for cid in range(N_CORES):
        cols = []
        for hh in range(NQ):
            qh = wq[:, cid * 512 + hh * 128: cid * 512 + (hh + 1) * 128]
            cols.append(qh[:, perm])
        kh = wq[:, H + cid * 128: H + (cid + 1) * 128]
        cols.append(kh[:, perm])
        vh = wq[:, H + 1024 + cid * 128: H + 1024 + (cid + 1) * 128]
        cols.append(vh)
        wcat = np.concatenate(cols, axis=1).astype(np.float16)
        in_maps.append({
            "xT": xT,
            "w": np.ascontiguousarray(wcat),
            "wo": np.ascontiguousarray(
                wo_f[cid * 512:(cid + 1) * 512, :].astype(np.float16)),
            "cs2": cs2,
            "sn2": sn2,
            "dmask": dmask,
            "swapm": swapm,
        })
    return in_maps


def kernel(x, last_pos, mask, rope_cache, wqkv, wo):
    global LAST_RESULTS
    from concourse.bass_utils import run_bass_kernel_spmd

    nc = _get_nc()
    in_maps = _prep_inputs(x, rope_cache, wqkv, wo)

    res = run_bass_kernel_spmd(nc, in_maps, list(range(N_CORES)))
    LAST_RESULTS = res
    if res.exec_time_ns is not None:
        print(f"HW exec time: {res.exec_time_ns} ns")
    yT = res.results[0]["yT"].astype(np.float64)
    for c in range(1, N_CORES):
        yT = yT + res.results[c]["yT"]
    return np.ascontiguousarray(yT.T).reshape(1, S, H).astype(np.float32)
